# revision 22
# baseline (speedup 1.0000x reference)
"""Trainium2 Bass kernel for nn_EquivariantModel (e3nn-style equivariant net).

Strategy: data-parallel over batch (8 cores x 1024 rows), feature-major
activations.  All o3.Linear layers (l1/l2, block out-linears, final) are
folded host-side into the FullyConnectedTensorProduct weights, so each block
reduces to a bilinear form in its RAW inputs:

    tp_s[b,w] = sum_{pq} s_p s_q MSS[pq,w] + sum_{i,pq} v_ip v_iq MVV[pq,w]
    tp_v[b,w,i] = sum_{pq} s_p v_iq MX[pq,w]

The symmetric forms (s(x)s, v(x)v) need only cyclic diagonals d=0..M/2 (2x
fewer products); products z_d = x * rot_d(x) are built with single
full-width DVE multiplies against partition-rotated copies of x, which are
materialized by grouped DMA reads from a row-doubled DRAM image (one DMA
covers many rotations via an overlapping-stride access pattern; the three
vector components share one image so a group is a single DMA).  Matmuls
stream z through per-diagonal weight tiles, accumulating in PSUM; the ss,
vv and sv streams are interleaved in one d-loop to keep DMA demand flat.
Block 1 (mult 64) stacks two batch halves in the partition dim and uses
split-K matmuls at base partitions 0/64.  A fraction of the products runs
on the GpSimd engine to unload the DVE.
"""

import sys
import numpy as np

if '/opt/trn_rl_repo' not in sys.path:
    sys.path.insert(0, '/opt/trn_rl_repo')

B, M_IN, M_HID = 8192, 64, 128
N_CORES = 8
BC = B // N_CORES            # batch per core
TANH_GAIN = 1.5927116870880127

GRP = 8                      # s-rotations / weight tiles per group DMA
POOL_EVERY = 0               # GpSimd muls serialize with DVE (shared SBUF
                             # port) — never offload products to GpSimd

_CACHE = {}


def _dedup_ldweights(nc):
    """Drop InstLdweights identical to the immediately preceding weight load
    (the PE array already holds those weights).  The h=0/h=1 matmul pair of
    every diagonal shares one weight tile, so half the loads are redundant;
    each exposed LDWEIGHTS costs ~107ns of PE serial time (full-128-row
    weight loads never overlap in-flight matmuls)."""
    removed = 0
    for blk in nc.m.functions[0].blocks:
        insts = blk.instructions
        prev = None
        i = 0
        while i < len(insts):
            x = insts[i]
            tn = type(x).__name__
            if tn == 'InstLdweights':
                a = x.ins[0]
                sig = (str(a.ap), a.offset, a.memref, str(a.dtype),
                       str(getattr(x, 'perf_mode', None)))
                si = x.sync_info
                clean = si is None or (not si.on_wait and not si.on_update)
                if prev == sig and clean:
                    del insts[i]
                    removed += 1
                    continue
                prev = sig
            elif tn == 'InstMatmult' and getattr(x, 'is_transpose', False):
                prev = None
            i += 1
    return removed


def _build_program(repeat=1):
    import concourse.mybir as mybir
    import concourse.tile as tile
    from concourse import bacc
    from contextlib import ExitStack
    import bass_rust

    f16 = mybir.dt.float16
    f32 = mybir.dt.float32

    nc = bacc.Bacc("TRN2", target_bir_lowering=False)

    # ---- DRAM I/O ----
    s2d = nc.dram_tensor("s2d", [128, BC], f16, kind="ExternalInput")
    v2da = nc.dram_tensor("v2da", [128, 3 * BC], f16, kind="ExternalInput")
    wss1 = nc.dram_tensor("wss1", [128, 17, 128], f16, kind="ExternalInput")
    wvv1 = nc.dram_tensor("wvv1", [128, 17, 128], f16, kind="ExternalInput")
    wsv1 = nc.dram_tensor("wsv1", [128, 32, 128], f16, kind="ExternalInput")
    wss2 = nc.dram_tensor("wss2", [128, 65, 128], f16, kind="ExternalInput")
    wvv2 = nc.dram_tensor("wvv2", [128, 65, 128], f16, kind="ExternalInput")
    wsv2 = nc.dram_tensor("wsv2", [128, 128, 128], f16, kind="ExternalInput")
    gate_w = {}
    for blk in ("1", "2"):
        for nm in ("s", "g", "v"):
            gate_w[blk + nm] = nc.dram_tensor(f"g{blk}{nm}", [128, 128], f16,
                                              kind="ExternalInput")
    wfs = nc.dram_tensor("wfs", [128, 64], f16, kind="ExternalInput")
    wfv = nc.dram_tensor("wfv", [128, 64], f16, kind="ExternalInput")

    dts = nc.dram_tensor("dts", [256, BC], f16, kind="Internal")
    dgva = nc.dram_tensor("dgva", [256, 3 * BC], f16, kind="Internal")
    out_d = nc.dram_tensor("out", [256, BC], f32, kind="ExternalOutput")

    def src_ap(t, dims, offset):
        s = t[:].copy()
        s.ap = bass_rust.VecI64Pair(dims)
        s.offset = offset
        return s

    with ExitStack() as ctx:
        tc = ctx.enter_context(tile.TileContext(nc))
        consts = ctx.enter_context(tc.tile_pool(name="consts", bufs=1))
        acts = ctx.enter_context(tc.tile_pool(name="acts", bufs=1))
        rotp = ctx.enter_context(tc.tile_pool(name="rot", bufs=2))
        rotv = ctx.enter_context(tc.tile_pool(name="rotv", bufs=2))
        rotb2 = ctx.enter_context(tc.tile_pool(name="rotb2", bufs=1))
        rotv2 = ctx.enter_context(tc.tile_pool(name="rotv2", bufs=2))
        wstr = ctx.enter_context(tc.tile_pool(name="wstr", bufs=2))
        wstv = ctx.enter_context(tc.tile_pool(name="wstv", bufs=2))
        zp1 = ctx.enter_context(tc.tile_pool(name="z1p", bufs=4))
        zp2 = ctx.enter_context(tc.tile_pool(name="z2p", bufs=4))
        psp = ctx.enter_context(tc.tile_pool(name="ps", bufs=1, space="PSUM"))
        tmp = ctx.enter_context(tc.tile_pool(name="tmp", bufs=1))

        GW = {}
        for k, t in gate_w.items():
            w = consts.tile([128, 128], f16, tag=f"gw{k}", name=f"gw{k}")
            nc.sync.dma_start(w[:], t[:])
            GW[k] = w
        wfs_sb = consts.tile([128, 64], f16, tag="wfs", name="wfs")
        nc.sync.dma_start(wfs_sb[:], wfs[:])
        wfv_sb = consts.tile([128, 64], f16, tag="wfv", name="wfv")
        nc.sync.dma_start(wfv_sb[:], wfv[:])

        nmul = [0]

        def mul(z, a, b):
            nmul[0] += 1
            if POOL_EVERY and nmul[0] % POOL_EVERY == 0:
                nc.gpsimd.tensor_mul(z, a, b)
            else:
                nc.vector.tensor_mul(z, a, b)

        def b1_block():
            # bases: features duplicated across both partition halves
            sb = acts.tile([128, BC], f16, tag="sb1", name="sb1")
            nc.sync.dma_start(sb[0:64, :], s2d[0:64, :])
            nc.sync.dma_start(sb[64:128, :], s2d[0:64, :])
            vba = acts.tile([128, 3 * BC], f16, tag="vb1", name="vb1")
            nc.sync.dma_start(vba[0:64, :], v2da[0:64, :])
            nc.sync.dma_start(vba[64:128, :], v2da[0:64, :])

            accs = psp.tile([128, 1024], f32, tag="pa_s", name="pa_s")
            accv = [psp.tile([128, 1024], f32, tag=f"pa_v{i}", name=f"pa_v{i}")
                    for i in range(3)]

            NPR = GRP // 2          # diagonal pairs per s-rot group

            # pair (2k, 2k+1): rot tile rows 0:64 = rot_2k, 64:128 = rot_2k+1
            for g0 in range(0, 64, GRP):
                npr = min(NPR, (64 - g0) // 2)
                rot = rotp.tile([128, NPR * BC], f16, tag="rotS1", name="rotS1")
                for h in range(2):
                    nc.sync.dma_start(
                        rot[h * 64:(h + 1) * 64, 0:npr * BC],
                        src_ap(s2d, [[BC, 64], [2 * BC, npr], [1, BC]],
                               (g0 + h) * BC))
                wsv_t = wstr.tile([128, NPR * 128], f16, tag="wsv", name="wsv")
                nc.scalar.dma_start(
                    wsv_t[:, 0:npr * 128],
                    wsv1[:, g0 // 2:g0 // 2 + npr, :].rearrange(
                        "p n m -> p (n m)"))
                nss = max(0, min(npr, 17 - g0 // 2))
                if nss > 0:
                    wss_t = wstr.tile([128, NPR * 128], f16, tag="wss", name="wss")
                    nc.scalar.dma_start(
                        wss_t[:, 0:nss * 128],
                        wss1[:, g0 // 2:g0 // 2 + nss, :].rearrange(
                            "p n m -> p (n m)"))
                # sv stream: wide mul per component over all pairs in group
                for i in range(3):
                    z = zp1.tile([128, NPR * BC], f16, tag="z1w", name="z1w",
                                 bufs=3)
                    mul(z[:, 0:npr * BC], rot[:, 0:npr * BC],
                        bcast(vba, i, npr))
                    for kp in range(npr):
                        gp = g0 // 2 + kp
                        for h in range(2):
                            nc.tensor.matmul(
                                accv[i][:, h * 512:(h + 1) * 512],
                                wsv_t[:, kp * 128:(kp + 1) * 128],
                                z[:, kp * BC + h * 512:kp * BC + (h + 1) * 512],
                                start=(gp == 0), stop=(gp == 31))
                # ss stream: one wide mul over the group's pairs
                if nss > 0:
                    z = zp1.tile([128, NPR * BC], f16, tag="z1w", name="z1w",
                                 bufs=3)
                    mul(z[:, 0:nss * BC], rot[:, 0:nss * BC],
                        bcast(sb, 0, nss))
                    for kp in range(nss):
                        gp = g0 // 2 + kp
                        for h in range(2):
                            nc.tensor.matmul(
                                accs[:, h * 512:(h + 1) * 512],
                                wss_t[:, kp * 128:(kp + 1) * 128],
                                z[:, kp * BC + h * 512:kp * BC + (h + 1) * 512],
                                start=(gp == 0), stop=False)
                # vv stream: one FD-3BC mul per pair (all 3 components)
                for kp in range(nss):
                    gp = g0 // 2 + kp
                    vr = rotv.tile([128, 3 * BC], f16, tag="rotV1",
                                   name="rotV1")
                    for hh in range(2):
                        nc.sync.dma_start(
                            vr[hh * 64:(hh + 1) * 64, :],
                            src_ap(v2da,
                                   [[3 * BC, 64], [BC, 3], [1, BC]],
                                   (2 * gp + hh) * 3 * BC))
                    wt = wstv.tile([128, 128], f16, tag="wvv", name="wvv")
                    nc.scalar.dma_start(
                        wt[:, :],
                        wvv1[:, gp:gp + 1, :].rearrange("p n m -> p (n m)"))
                    z = zp1.tile([128, 4 * BC], f16, tag="z1w", name="z1w",
                                 bufs=3)
                    mul(z[:, 0:3 * BC], vba, vr)
                    for i in range(3):
                        for h in range(2):
                            nc.tensor.matmul(
                                accs[:, h * 512:(h + 1) * 512], wt[:, :],
                                z[:, i * BC + h * 512:i * BC + (h + 1) * 512],
                                start=False,
                                stop=(gp == 16 and i == 2))
            return accs, accv

        def bcast(t, col, n):
            """Stride-0 broadcast AP: repeat tile column-block [col, col+BC)
            n times along the free dim."""
            s = t[:, col * BC:(col + 1) * BC]
            b = s.copy()
            b.ap = bass_rust.VecI64Pair([list(s.ap[0]), [0, n], [1, BC]])
            return b

        def b2_block():
            """Difference-basis b2: products of pre-rotated tiles.

            SRf/VRf hold rotations 0..7, SRc/VRc rotations 8,16,..,64 of the
            s / v activations (read from the doubled DRAM images).  Diagonal
            d = 8k - j of the symmetric streams is SR_j * SR_8k; the sv
            stream gets c = 8k - j from SRc_k * VRf_j and c = 128 - 8k + j
            from SRf_j * VRc_k.  Muls are 4 diagonals wide (one operand
            broadcast).  Per-component pass order ss / sv-K1 / vv / sv-K2
            hides the single-buffered VRf/VRc reloads behind compute.
            Weight slot layouts match _host_prep."""
            accs = psp.tile([128, 1024], f32, tag="pa_s", name="pa_s")
            accv = [psp.tile([128, 1024], f32, tag=f"pa_v{i}", name=f"pa_v{i}")
                    for i in range(3)]

            SRf = rotb2.tile([128, 8 * BC], f16, tag="SRf", name="SRf")
            nc.sync.dma_start(
                SRf[:, :], src_ap(dts, [[BC, 128], [BC, 8], [1, BC]], 0))
            SRc = rotb2.tile([128, 8 * BC], f16, tag="SRc", name="SRc")
            nc.sync.dma_start(
                SRc[:, :], src_ap(dts, [[BC, 128], [8 * BC, 8], [1, BC]],
                                  8 * BC))

            def mm(acc, w, z, zoff, start, stop):
                for h in range(2):
                    nc.tensor.matmul(acc[:, h * 512:(h + 1) * 512], w,
                                     z[:, zoff + h * 512:zoff + (h + 1) * 512],
                                     start=start, stop=stop)

            def quad(acc, wk, wslot0, z, start, stop):
                # z holds 4 diagonals' products; contract each against its
                # weight slot (wslot0..wslot0+3 within tile wk)
                for jj in range(4):
                    mm(acc, wk[:, (wslot0 + jj) * 128:(wslot0 + jj + 1) * 128],
                       z, jj * BC, start and jj == 0, stop and jj == 3)

            for i in range(3):
                VRf = rotv2.tile([128, 8 * BC], f16, tag="VRf", name="VRf",
                                 bufs=1)
                nc.sync.dma_start(
                    VRf[:, :],
                    src_ap(dgva, [[3 * BC, 128], [3 * BC, 8], [1, BC]],
                           i * BC))
                VRc = rotv2.tile([128, 8 * BC], f16, tag="VRc", name="VRc",
                                 bufs=1)
                nc.sync.dma_start(
                    VRc[:, :],
                    src_ap(dgva, [[3 * BC, 128], [24 * BC, 8], [1, BC]],
                           24 * BC + i * BC))

                if i == 0:
                    # ss stream -> accs (SR tiles only)
                    w0 = wstr.tile([128, 128], f16, tag="wss0", name="wss0",
                                   bufs=1)
                    nc.scalar.dma_start(
                        w0[:, :], wss2[:, 0:1, :].rearrange("p n m -> p (n m)"))
                    z = zp2.tile([128, BC], f16, tag="z2", name="z2", bufs=1)
                    mul(z, SRf[:, 0:BC], SRf[:, 0:BC])
                    mm(accs, w0[:, :], z, 0, True, False)
                    for k in range(1, 9):
                        wk = wstr.tile([128, 8 * 128], f16, tag="wss",
                                       name="wss")
                        nc.scalar.dma_start(
                            wk[:, :], wss2[:, 8 * k - 7:8 * k + 1, :]
                            .rearrange("p n m -> p (n m)"))
                        for jh in range(2):
                            z = zp2.tile([128, 4 * BC], f16, tag="z2w",
                                         name="z2w", bufs=3)
                            mul(z, SRf[:, 4 * jh * BC:(4 * jh + 4) * BC],
                                bcast(SRc, k - 1, 4))
                            quad(accs, wk, 4 * jh, z, False, False)

                # sv c0 + K1 -> accv[i]  (VRf x SRc; VRc not needed yet)
                w0 = wstr.tile([128, 128], f16, tag="wsv0", name="wsv0",
                               bufs=1)
                nc.scalar.dma_start(
                    w0[:, :], wsv2[:, 0:1, :].rearrange("p n m -> p (n m)"))
                z = zp2.tile([128, BC], f16, tag="z2", name="z2", bufs=1)
                mul(z, SRf[:, 0:BC], VRf[:, 0:BC])
                mm(accv[i], w0[:, :], z, 0, True, False)
                for k in range(1, 9):
                    wk = wstr.tile([128, 8 * 128], f16, tag="wsvA",
                                   name="wsvA")
                    nc.scalar.dma_start(
                        wk[:, :], wsv2[:, 8 * k - 7:8 * k + 1, :]
                        .rearrange("p n m -> p (n m)"))
                    for jh in range(2):
                        z = zp2.tile([128, 4 * BC], f16, tag="z2w",
                                     name="z2w", bufs=3)
                        mul(z, VRf[:, 4 * jh * BC:(4 * jh + 4) * BC],
                            bcast(SRc, k - 1, 4))
                        quad(accv[i], wk, 4 * jh, z, False, False)

                # vv stream -> accs (first VRc consumer)
                w0 = wstv.tile([128, 128], f16, tag="wvv0", name="wvv0",
                               bufs=1)
                nc.scalar.dma_start(
                    w0[:, :], wvv2[:, 0:1, :].rearrange("p n m -> p (n m)"))
                z = zp2.tile([128, BC], f16, tag="z2", name="z2", bufs=1)
                mul(z, VRf[:, 0:BC], VRf[:, 0:BC])
                mm(accs, w0[:, :], z, 0, False, False)
                for k in range(1, 9):
                    wk = wstv.tile([128, 8 * 128], f16, tag="wvv", name="wvv")
                    nc.scalar.dma_start(
                        wk[:, :], wvv2[:, 8 * k - 7:8 * k + 1, :]
                        .rearrange("p n m -> p (n m)"))
                    for jh in range(2):
                        z = zp2.tile([128, 4 * BC], f16, tag="z2w",
                                     name="z2w", bufs=3)
                        mul(z, VRf[:, 4 * jh * BC:(4 * jh + 4) * BC],
                            bcast(VRc, k - 1, 4))
                        quad(accs, wk, 4 * jh, z, False,
                             i == 2 and k == 8 and jh == 1)

                # sv K2 -> accv[i]: c = 128-8k+j, roll 8k; slots 65..
                # (k-major, (8,0) skipped).  VRf free after this loop's
                # first half; its reload for pass i+1 overlaps K2.
                for k in range(1, 9):
                    ns = 8 if k < 8 else 7
                    base = 65 + 8 * (k - 1)
                    j0 = 0 if k < 8 else 1
                    wk = wstr.tile([128, 8 * 128], f16, tag="wsvA",
                                   name="wsvB")
                    nc.scalar.dma_start(
                        wk[:, 0:ns * 128],
                        wsv2[:, base:base + ns, :]
                        .rearrange("p n m -> p (n m)"))
                    for jh in range(2):
                        jstart = j0 + 4 * jh if k < 8 else (1 + 3 * jh)
                        nj = 4 if k < 8 else (3 if jh == 0 else 4)
                        z = zp2.tile([128, 4 * BC], f16, tag="z2w",
                                     name="z2w", bufs=3)
                        mul(z[:, 0:nj * BC],
                            SRf[:, jstart * BC:(jstart + nj) * BC],
                            bcast(VRc, k - 1, nj))
                        for jj in range(nj):
                            widx = (jstart - j0) + jj
                            mm(accv[i],
                               wk[:, widx * 128:(widx + 1) * 128],
                               z, jj * BC, False,
                               k == 8 and jh == 1 and jj == nj - 1)
            return accs, accv

        def gate(blk, accs, accv, dup=False):
            """PSUM accs -> (tanh_s, gated_v[3]) f16; dup doubles the free dim
            (cols BC:2BC replicate 0:BC) for paired-diagonal consumption."""
            W_ = 2 * BC if dup else BC
            tp_s = acts.tile([128, BC], f16, tag="tps", name="tps")
            nc.scalar.copy(tp_s[:, :], accs[:, :])
            tp_v = []
            for i in range(3):
                t = acts.tile([128, BC], f16, tag=f"tpv{i}", name=f"tpv{i}")
                nc.scalar.copy(t[:, :], accv[i][:, :])
                tp_v.append(t)
            tanh_s = acts.tile([128, W_], f16, tag=f"ths{blk}", name=f"ths{blk}")
            tg = acts.tile([128, BC], f16, tag="tg", name="tg")
            vl = [acts.tile([128, BC], f16, tag=f"vl{i}", name=f"vl{i}")
                  for i in range(3)]
            # gate matmuls reuse the freed accumulator PSUM slots
            ps = psp.tile([128, 1024], f32, tag="pa_s", name="pa_s")
            psg = psp.tile([128, 1024], f32, tag="pa_v0", name="pa_v0")
            psv = [psp.tile([128, 1024], f32, tag=f"pa_v{i}", name=f"pa_v{i}")
                   for i in (1, 2)]
            psv.append(psp.tile([128, 1024], f32, tag="pa_s", name="pa_s"))
            for h in range(2):
                sl_ = slice(h * 512, (h + 1) * 512)
                nc.tensor.matmul(ps[:, sl_], GW[blk + "s"], tp_s[:, sl_],
                                 start=True, stop=True)
                nc.scalar.activation(tanh_s[:, sl_], ps[:, sl_],
                                     mybir.ActivationFunctionType.Tanh)
                nc.tensor.matmul(psg[:, sl_], GW[blk + "g"], tp_s[:, sl_],
                                 start=True, stop=True)
                nc.scalar.activation(tg[:, sl_], psg[:, sl_],
                                     mybir.ActivationFunctionType.Tanh)
                for i in range(3):
                    nc.tensor.matmul(psv[i][:, sl_], GW[blk + "v"],
                                     tp_v[i][:, sl_], start=True, stop=True)
                    nc.scalar.copy(vl[i][:, sl_], psv[i][:, sl_])
            gated = []
            for i in range(3):
                t = acts.tile([128, W_], f16, tag=f"gv{blk}{i}", name=f"gv{blk}{i}")
                nc.vector.tensor_mul(t[:, 0:BC], tg, vl[i])
                if dup:
                    nc.scalar.copy(t[:, BC:2 * BC], t[:, 0:BC])
                gated.append(t)
            if dup:
                nc.scalar.copy(tanh_s[:, BC:2 * BC], tanh_s[:, 0:BC])
            return tanh_s, gated

        def _network():
            accs, accv = b1_block()
            tanh_s1, gated1 = gate("1", accs, accv)
            # doubled DRAM images for b2 rotations
            nc.sync.dma_start(dts[0:128, :], tanh_s1[:, 0:BC])
            nc.sync.dma_start(dts[128:256, :], tanh_s1[:, 0:BC])
            for i in range(3):
                nc.sync.dma_start(dgva[0:128, i * BC:(i + 1) * BC],
                                  gated1[i][:, 0:BC])
                nc.sync.dma_start(dgva[128:256, i * BC:(i + 1) * BC],
                                  gated1[i][:, 0:BC])
            accs2, accv2 = b2_block()
            tanh_s2, gated2 = gate("2", accs2, accv2)
            # final linears (out-linears folded in)
            fps = psp.tile([128, 1024], f32, tag="pa_v0", name="pa_v0")
            fpv = [psp.tile([128, 1024], f32, tag=t, name=t)
                   for t in ("pa_v1", "pa_v2", "pa_s")]
            for h in range(2):
                sl_ = slice(h * 512, (h + 1) * 512)
                nc.tensor.matmul(fps[0:64, sl_], wfs_sb[:], tanh_s2[:, sl_],
                                 start=True, stop=True)
                ot = tmp.tile([64, 512], f32, tag="ot", name="ot", bufs=2)
                nc.scalar.copy(ot[:, :], fps[0:64, sl_])
                nc.scalar.dma_start(out_d[0:64, sl_], ot[:])
                for i in range(3):
                    nc.tensor.matmul(fpv[i][0:64, sl_], wfv_sb[:],
                                     gated2[i][:, sl_], start=True, stop=True)
                    ov = tmp.tile([64, 512], f32, tag="ot", name="ot", bufs=2)
                    nc.scalar.copy(ov[:, :], fpv[i][0:64, sl_])
                    nc.scalar.dma_start(out_d[64 + 64 * i:128 + 64 * i, sl_], ov[:])

        if repeat > 1:
            with tc.For_i(0, repeat, 1):
                _network()
        else:
            _network()

    _dedup_ldweights(nc)
    nc.finalize()
    return nc


def _host_prep(inputs):
    """Fold all linears into TP weights; build diagonal weight streams."""
    f = {k: np.asarray(v, np.float64) for k, v in inputs.items() if k != 'x'}
    d = {}
    fold = {}
    for blk, M in (("b1", 64), ("b2", 128)):
        c1 = 1.0 / np.sqrt(M)
        A, Av = f[f"{blk}_l1_w0"] * c1, f[f"{blk}_l1_w1"] * c1
        Bm, Bv = f[f"{blk}_l2_w0"] * c1, f[f"{blk}_l2_w1"] * c1
        if blk == "b2":
            A, Av = fold["O1s"] @ A, fold["O1v"] @ Av
            Bm, Bv = fold["O1s"] @ Bm, fold["O1v"] @ Bv
        ctp = 1.0 / (M * np.sqrt(2.0))

        def fld(L, R, W, c):
            T = np.tensordot(L, W, axes=(1, 0))
            T = np.tensordot(R, T, axes=(1, 1))
            return c * T.transpose(1, 0, 2)
        MSS = fld(A, Bm, f[f"{blk}_tp_ss"], ctp)
        MVV = fld(Av, Bv, f[f"{blk}_tp_vv"], ctp / np.sqrt(3.0))
        MSV = fld(A, Bv, f[f"{blk}_tp_sv"], ctp)
        MVS = fld(Av, Bm, f[f"{blk}_tp_vs"], ctp)
        MX = MSV + MVS.transpose(1, 0, 2)
        Min = A.shape[0]
        ar = np.arange(Min)
        sym = {}
        for nm, Msym in (("ss", MSS), ("vv", MVV)):
            tiles = []
            for dd in range(Min // 2 + 1):
                idx = (ar + dd) % Min
                if dd == 0:
                    w = Msym[ar, ar, :]
                elif dd == Min // 2:
                    w = (Msym[ar, idx, :] + Msym[idx, ar, :]) * 0.5
                else:
                    w = Msym[ar, idx, :] + Msym[idx, ar, :]
                tiles.append(w)
            sym[nm] = np.stack(tiles, axis=1)                 # [Min, nd, 128]
        rect = np.stack([MX[(ar + c) % Min, ar, :] for c in range(Min)],
                        axis=1)                               # [Min, Min, 128]
        if blk == "b2":
            # Difference-basis reorg: products come from pairs of rotated
            # tiles; diagonal d = 8k - j uses (rot_j, rot_8k), so the weight
            # rows are pre-rolled by the first offset (ss/vv: j; sv K1: j,
            # K2: 8k).  Slot layout (second axis):
            #   sym:  [0]=d0, [1+8(k-1)+j] = diag 8k-j rolled by j
            #   rect: [0]=c0, [1..64] = K1 (c=8k-j, roll j),
            #         [65..127] = K2 (c=128-8k+j, roll 8k; (k=8,j=0) skipped)
            M2 = Min
            qar = np.arange(M2)

            def reorg_sym(wD):
                out = np.zeros((M2, 65, 128), np.float64)
                out[:, 0, :] = wD[:, 0, :]
                for k in range(1, 9):
                    for j in range(8):
                        out[:, 8 * k - 7 + j, :] = wD[(qar + j) % M2,
                                                      8 * k - j, :]
                return out

            sym["ss"] = reorg_sym(sym["ss"])
            sym["vv"] = reorg_sym(sym["vv"])
            rnew = np.zeros((M2, 128, 128), np.float64)
            rnew[:, 0, :] = rect[:, 0, :]
            for k in range(1, 9):
                for j in range(8):
                    rnew[:, 8 * k - 7 + j, :] = rect[(qar + j) % M2,
                                                     8 * k - j, :]
            slot = 65
            for k in range(1, 9):
                for j in range(8):
                    if k == 8 and j == 0:
                        continue
                    rnew[:, slot, :] = rect[(qar + 8 * k) % M2,
                                            128 - 8 * k + j, :]
                    slot += 1
            rect = rnew
        if blk == "b1":
            # pack diagonal pairs (2k, 2k+1) into 128-row tiles; odd counts
            # get a zero-padded bottom half
            def pairs(st):
                nd = st.shape[1]
                tiles = []
                for k in range((nd + 1) // 2):
                    top = st[:, 2 * k, :]
                    bot = (st[:, 2 * k + 1, :] if 2 * k + 1 < nd
                           else np.zeros_like(top))
                    tiles.append(np.concatenate([top, bot], axis=0))
                return np.stack(tiles, axis=1)                # [128, np, 128]
            for nm in ("ss", "vv"):
                sym[nm] = pairs(sym[nm])
            rect = pairs(rect)
        bn = blk[1]
        d[f"wss{bn}"] = np.ascontiguousarray(sym["ss"]).astype(np.float16)
        d[f"wvv{bn}"] = np.ascontiguousarray(sym["vv"]).astype(np.float16)
        d[f"wsv{bn}"] = np.ascontiguousarray(rect).astype(np.float16)
        cg = 1.0 / np.sqrt(128)
        for nm, sfx in (("ws", "s"), ("wg", "g"), ("wv", "v")):
            d[f"g{bn}{sfx}"] = (f[f"{blk}_g_{nm}"] * cg).astype(np.float16)
        cog = TANH_GAIN / np.sqrt(128)
        fold[f"O{bn}s"] = f[f"{blk}_o_w0"] * cog
        fold[f"O{bn}v"] = f[f"{blk}_o_w1"] * cog
    cf = 1.0 / np.sqrt(128)
    d["wfs"] = (fold["O2s"] @ (f["f_w0"] * cf)).astype(np.float16)
    d["wfv"] = (fold["O2v"] @ (f["f_w1"] * cf)).astype(np.float16)
    return d


def _make_in_maps(x, w):
    x = np.asarray(x, dtype=np.float32)
    in_maps = []
    for c in range(N_CORES):
        bs = slice(c * BC, (c + 1) * BC)
        xl = x[bs]
        s_loc = np.ascontiguousarray(xl[:, :64].T).astype(np.float16)   # [64, BC]
        v_loc = xl[:, 64:].reshape(BC, 64, 3)
        m = dict(w)
        m["s2d"] = np.concatenate([s_loc, s_loc], axis=0)
        va = np.concatenate([np.ascontiguousarray(v_loc[:, :, i].T)
                             .astype(np.float16) for i in range(3)], axis=1)
        m["v2da"] = np.concatenate([va, va], axis=0)          # [128, 3*BC]
        in_maps.append(m)
    return in_maps


def kernel(**inputs):
    from concourse.bass_utils import run_bass_kernel_spmd

    w = _host_prep(inputs)
    in_maps = _make_in_maps(inputs["x"], w)

    if "nc" not in _CACHE:
        _CACHE["nc"] = _build_program()
    nc = _CACHE["nc"]

    res = run_bass_kernel_spmd(nc, in_maps, core_ids=list(range(N_CORES)))

    out = np.empty((B, 256), dtype=np.float32)
    for c in range(N_CORES):
        o = res.results[c]["out"]                                # [256, BC]
        bs = slice(c * BC, (c + 1) * BC)
        out[bs, :64] = o[:64].T
        v = o[64:].reshape(3, 64, BC)
        out[bs, 64:] = v.transpose(2, 1, 0).reshape(BC, 192)
    return out

